# revision 41
# baseline (speedup 1.0000x reference)
"""Trainium2 Bass kernel for nn_MultiHeadHighLevelAllocator.

Math (reference):
    ue = MLP3(uav_feat)                            # (B,U,E)
    te = MLP3(task_feat)                           # (B,T,E)
    q  = ue[:,None,:,:] + head_q[None,:,None,:]    # (B,H,U,E)
    logits[b,h,u,t] = relu(q[b,h,u]@Wq + te[b,t]@Wk + fb1) @ fw2 + fb2

Key decomposition: by linearity of the projections,
    pre[b,h,u,t,:] = base[b,u,t,:] + hqP[h,:]
where base[b,u,t,:] = ue[b,u]@Wq + te[b,t]@Wk  (outer sum, H-independent)
and   hqP[h,:] = head_q[h]@Wq + fb1.

Per-core (data parallel over B, 2 batches/core), per (b, c-chunk of HID):
  1. Encoders on TensorE; ReLU+bias on DVE (ue chain) / ScalarE (te chain).
  2. khP = Wk te (128,T) and qp8 = Wq ue with each u-column repeated 8x
     (stride-0 moving AP on the projection matmul), evicted fp16.
  3. base slab (128, U*T) built by ONE DVE tensor_tensor in 2x_1p mode:
     in0 = khP broadcast over u (inner stride-1 runs of 8 keep the mode),
     in1 = qp8 re-read via [u:8x64][rep:0x16][r:1x8].
  4. Per head h: rt = relu(base + hqP[h]) via big tensor_scalar (DVE 4x
     mode) / ScalarE activation ops; ScalarE's share is capped to what
     fits in its per-section window so it never gates the matmul waves.
  5. Reduction via masked-stationary matmuls: (128x32) fp16 stationary
     holding fw2-chunk in column j writes the dot product to PSUM
     partition 32g+j of strip g's bank; subwaves of 2 strips issue
     round-robin so col-strips stream concurrently.
  6. Per-strip +fb2 eviction as soon as that strip's accumulation stops
     (g0/g2 after the h1 wave), then phased DMA out.

Inputs are packed host-side into three fp16 tensors (one per DMA phase,
each a contiguous whole-tensor copy) plus a small fp32 bias tensor.
"""
import os
import sys

for _p in ("/opt/trn_rl_repo", "/root/.axon_site/_ro/trn_rl_repo"):
    if os.path.isdir(_p) and _p not in sys.path:
        sys.path.insert(0, _p)

import numpy as np
import concourse.bass as bass
import concourse.mybir as mybir
from concourse import tile

B, U, T = 16, 64, 128
UAV_DIM, TASK_DIM = 32, 32
E, H, HID = 128, 4, 256
ENC_H = 128
NCORES = 8
BL = B // NCORES          # batches per core
NBLK = U // 4             # 16 u-blocks of 4 us -> N=512 columns each
f32, f16 = mybir.dt.float32, mybir.dt.float16
AF = mybir.ActivationFunctionType
ALU = mybir.AluOpType

# phase-1 fp16 tensor: encoder inputs + layer-1 weights
_A_UAVT = 0          # (32, 128)
_A_TASKT = 128       # (32, 256)
_A_UW0 = 384         # (32, 128)
_A_TW0 = 512         # (32, 128)
_A_TOT = 640
# phase-2 fp16 tensor: layer-2/3 encoder weights
_B_UW1 = 0
_B_UW2 = 128
_B_TW1 = 256
_B_TW2 = 384
_B_TOT = 512
# phase-3 fp16 tensor: projection weights + masked fw2 windows
_C_WQK = 0           # (128, 512): Wq c0 | Wq c1 | Wk c0 | Wk c1
_C_WZ = 512          # (128, 126); fw2 chunk c at col c*63+31
_C_TOT = 638
# fp32 tensor: 7 encoder biases + fb2, then 8 hqP columns
_G_ENCB = 0
_G_HQPB = 7
_G_TOT = 15

_cache: dict = {}


def _split_multi_waits(nc):
    """Walrus in this toolchain rejects >1 sync wait per engine instruction
    ("Too many sync wait commands"). Hoist extra waits onto preceding
    same-engine NoOps — identical semantics on the in-order engine queues."""
    n_split = 0
    for func in nc.m.functions:
        for bb in func.blocks:
            new = []
            for ins in bb.instructions:
                si = ins.sync_info
                waits = list(si.on_wait) if (si and si.on_wait) else []
                if len(waits) > 1:
                    for k, w in enumerate(waits[:-1]):
                        nop = mybir.InstNoOp(name=f"{ins.name}_hw{k}", ins=[], outs=[])
                        nop.engine = ins.engine
                        nop.sync_info = mybir.SyncInfo(on_wait=[w], on_update=[])
                        new.append(nop)
                        n_split += 1
                    si.on_wait = [waits[-1]]
                new.append(ins)
            bb.instructions = new
    return n_split


def _build_nc():
    nc = bass.Bass()
    pA = nc.dram_tensor("pA", [128, _A_TOT], f16, kind="ExternalInput")
    pB = nc.dram_tensor("pB", [128, _B_TOT], f16, kind="ExternalInput")
    pC = nc.dram_tensor("pC", [128, _C_TOT], f16, kind="ExternalInput")
    pG = nc.dram_tensor("pG", [128, _G_TOT], f32, kind="ExternalInput")
    out = nc.dram_tensor("out", [128, 512], f32, kind="ExternalOutput")

    with tile.TileContext(nc) as tc:
        with (
            tc.tile_pool(name="const", bufs=1) as constp,
            tc.tile_pool(name="persist", bufs=1) as persistp,
            tc.tile_pool(name="encw", bufs=2) as encwp,
        ):
            A = constp.tile([128, _A_TOT], f16, tag="a16")
            G = constp.tile([128, _G_TOT], f32, tag="g32")
            Bt = constp.tile([128, _B_TOT], f16, tag="b16")
            C = constp.tile([128, _C_TOT], f16, tag="c16")
            # two HWDGE queues (sync + scalar) load the phases in parallel
            nc.sync.dma_start(A[:], pA[:])
            nc.scalar.dma_start(G[:], pG[:])
            nc.sync.dma_start(Bt[:], pB[:])
            nc.scalar.dma_start(C[:], pC[:])
            # first touches per engine so later ops never pair a DMA-sem wait
            # with an engine-sem wait in one instruction
            act_touch = constp.tile([128, 1], f32, tag="acttouch")
            nc.scalar.copy(act_touch[:], G[:, 0:1])
            dve_touch = constp.tile([128, 1], f32, tag="dvetouch")
            nc.vector.tensor_copy(dve_touch[:], G[:, 0:1])
            act_touch2 = constp.tile([128, 1], f16, tag="acttouch2")
            nc.scalar.copy(act_touch2[:], A[:, 0:1])
            dve_touch2 = constp.tile([128, 1], f16, tag="dvetouch2")
            nc.vector.tensor_copy(dve_touch2[:], A[:, 0:1])

            enc_w = {
                "uw0": A[0:32, _A_UW0:_A_UW0 + 128],
                "tw0": A[0:32, _A_TW0:_A_TW0 + 128],
                "uw1": Bt[:, _B_UW1:_B_UW1 + 128],
                "uw2": Bt[:, _B_UW2:_B_UW2 + 128],
                "tw1": Bt[:, _B_TW1:_B_TW1 + 128],
                "tw2": Bt[:, _B_TW2:_B_TW2 + 128],
            }

            def encb_col(i):
                return G[:, _G_ENCB + i:_G_ENCB + i + 1]

            def hqp_col(c, h):
                i = _G_HQPB + c * 4 + h
                return G[:, i:i + 1]

            with (
                tc.tile_pool(name="bsbp", bufs=4) as bsbp,
                tc.tile_pool(name="relup16", bufs=5) as relup16,
                tc.tile_pool(name="relup8", bufs=2) as relup8,
                tc.tile_pool(name="outp", bufs=1) as outp,
                tc.tile_pool(name="bpp", bufs=4, space="PSUM") as psB,
                tc.tile_pool(name="lpp", bufs=1, space="PSUM") as psL,
            ):
                relups = {16: relup16, 8: relup8}
                # ---- encoders: ue acts on DVE, te acts on ScalarE so the
                #      two chains run in parallel ----
                chains = {
                    "ue": [A[0:32, _A_UAVT:_A_UAVT + BL * U], BL * U,
                           ("uw0", "uw1", "uw2"), (0, 1, 2)],
                    "te": [A[0:32, _A_TASKT:_A_TASKT + BL * T], BL * T,
                           ("tw0", "tw1", "tw2"), (3, 4, 5)],
                }
                cur = {k: v[0] for k, v in chains.items()}
                for li in range(3):
                    pss = {}
                    for k, (x0, rows, wn, bc) in chains.items():
                        ps = psB.tile([128, 512], f32, tag="bp",
                                      name=f"ps{k}{li}")
                        nc.tensor.matmul(ps[:, :rows], enc_w[wn[li]], cur[k],
                                         start=True, stop=True)
                        pss[k] = ps
                    for k, (x0, rows, wn, bc) in chains.items():
                        pool = encwp if li < 2 else persistp
                        nxt = pool.tile([128, rows], f16, tag=f"{k}h{li}",
                                        name=f"{k}h{li}")
                        if k == "ue":
                            if li < 2:
                                nc.vector.tensor_scalar(
                                    nxt[:], pss[k][:, :rows],
                                    encb_col(bc[li]), 0.0, ALU.add, ALU.max)
                            else:
                                nc.vector.tensor_scalar(
                                    nxt[:], pss[k][:, :rows],
                                    encb_col(bc[li]), None, ALU.add)
                        else:
                            nc.scalar.activation(
                                nxt[:], pss[k][:, :rows],
                                AF.Relu if li < 2 else AF.Identity,
                                bias=encb_col(bc[li]), scale=1.0)
                        cur[k] = nxt[:]
                ueT, teT = cur["ue"], cur["te"]

                # ---- projections for all (b,c): khP (128,T) f16 and
                #      qp8 (128,512) f16 = Wq ue with u-cols repeated 8x;
                #      (c0,b0) eviction on DVE (feeds its own first TT),
                #      the rest on ScalarE ----
                khPs, qp8s = {}, {}
                for c in range(2):
                    for b in range(BL):
                        pk = psB.tile([128, 512], f32, tag="bp",
                                      name=f"pk{b}{c}")
                        nc.tensor.matmul(pk[:, :T],
                                         C[:, _C_WQK + 256 + c * 128:
                                           _C_WQK + 256 + (c + 1) * 128],
                                         teT[:, b * T:(b + 1) * T],
                                         start=True, stop=True)
                        pq = psB.tile([128, 512], f32, tag="bp",
                                      name=f"pq{b}{c}")
                        mov = ueT[:, b * U:(b + 1) * U].unsqueeze(2) \
                            .to_broadcast([128, U, 8])
                        nc.tensor.matmul(pq[:].rearrange("p (u r) -> p u r", r=8),
                                         C[:, _C_WQK + c * 128:
                                           _C_WQK + (c + 1) * 128],
                                         mov, start=True, stop=True)
                        khP = persistp.tile([128, T], f16, tag=f"khP{b}{c}",
                                            name=f"khP{b}{c}")
                        qp8 = persistp.tile([128, 512], f16, tag=f"qp8{b}{c}",
                                            name=f"qp8{b}{c}")
                        if c == 0 and b == 0:
                            # split across engines: shortest path to the
                            # first build
                            nc.vector.tensor_copy(khP[:], pk[:, :T])
                            nc.scalar.copy(qp8[:], pq[:])
                        else:
                            nc.scalar.copy(khP[:], pk[:, :T])
                            nc.scalar.copy(qp8[:], pq[:])
                        khPs[(b, c)], qp8s[(b, c)] = khP, qp8

                lp = [psL.tile([128, 512], f32, tag=f"lp{g}", name=f"lp{g}")
                      for g in range(4)]
                sb_out = outp.tile([128, 512], f32, tag="sbout", name="sbout")

                def emit_build(c, b, halves=False):
                    bsb = bsbp.tile([128, U * T], f16, tag="bsb",
                                    name=f"bsb{b}{c}")
                    khP, qp8 = khPs[(b, c)], qp8s[(b, c)]
                    pieces = ((0, 32), (32, 64)) if halves else ((0, 64),)
                    for (ul, uh) in pieces:
                        nu = uh - ul
                        dst = bsb[:, ul * T:uh * T].rearrange(
                            "p (u v r) -> p u v r", v=16, r=8)
                        in0 = khP[:].rearrange("p (v r) -> p v r", r=8) \
                            .unsqueeze(1).to_broadcast([128, nu, 16, 8])
                        in1 = qp8[:, ul * 8:uh * 8] \
                            .rearrange("p (u r) -> p u r", r=8) \
                            .unsqueeze(2).to_broadcast([128, nu, 16, 8])
                        nc.vector.tensor_tensor(dst, in0, in1, ALU.add)
                    return bsb

                def rt_piece(c, b, h, bsb, eng, nlo, nhi):
                    rt = relups[nhi - nlo].tile(
                        [128, (nhi - nlo) * 512], f16,
                        tag=f"rt{nhi - nlo}", name=f"rt{c}{b}{h}_{nlo}")
                    lo, hi = nlo * 512, nhi * 512
                    if eng == 'S':
                        nc.scalar.activation(rt[:], bsb[:, lo:hi], AF.Relu,
                                             bias=hqp_col(c, h), scale=1.0)
                    else:
                        nc.vector.tensor_scalar(rt[:], bsb[:, lo:hi],
                                                hqp_col(c, h), 0.0,
                                                ALU.add, ALU.max)
                    return (nlo, nhi, rt)

                def emit_subwave(c, pair, n_order=None):
                    # pair: list of (b, h, [(n_lo, n_hi, rt_tile), ...]);
                    # n_order permutes the u-block issue order so a
                    # late-finishing producer's blocks can issue last
                    if n_order is None:
                        n_order = range(NBLK)
                    n_order = list(n_order)
                    for n in n_order:
                        for (b, h, pieces) in pair:
                            for (n_lo, n_hi, t_) in pieces:
                                if n_lo <= n < n_hi:
                                    rt, off = t_, n - n_lo
                                    break
                            p_ = (b * H + h) * NBLK + n
                            g, j = p_ // 32, p_ % 32
                            first = (c == 0 and n == 0 and h % 2 == 0)
                            last = (c == 1 and n == n_order[-1]
                                    and h % 2 == 1)
                            nc.tensor.matmul(
                                lp[g][32 * g:32 * g + 32, :],
                                C[:, _C_WZ + c * 63 + 31 - j:
                                  _C_WZ + c * 63 + 63 - j],
                                rt[:, off * 512:(off + 1) * 512],
                                start=first, stop=last,
                                tile_position=(0, 32 * g))

                def emit_evict(g):
                    nc.scalar.activation(
                        sb_out[32 * g:32 * g + 32, :],
                        lp[g][32 * g:32 * g + 32, :],
                        AF.Identity,
                        bias=G[32 * g:32 * g + 32, _G_ENCB + 6:_G_ENCB + 7],
                        scale=1.0)

                for c in range(2):
                    # ScalarE chain per section: (b0,h0) whole, (b0,h1)
                    # whole, (b0,h3) first half — everything else DVE.
                    # Subwave order h0, h2, h1, h3; within the h3 subwave
                    # DVE's u-blocks issue first so ScalarE's half (the
                    # last producer to finish) never stalls the PE queue.
                    bsb0 = emit_build(c, 0)
                    sc_h0 = rt_piece(c, 0, 0, bsb0, 'S', 0, NBLK)
                    bsb1 = emit_build(c, 1)
                    d_b1h0 = rt_piece(c, 1, 0, bsb1, 'D', 0, NBLK)
                    emit_subwave(c, [(0, 0, [sc_h0]), (1, 0, [d_b1h0])])
                    d_b0h2 = rt_piece(c, 0, 2, bsb0, 'D', 0, NBLK)
                    d_b1h2 = rt_piece(c, 1, 2, bsb1, 'D', 0, NBLK)
                    emit_subwave(c, [(0, 2, [d_b0h2]), (1, 2, [d_b1h2])])
                    sc_h1 = rt_piece(c, 0, 1, bsb0, 'S', 0, NBLK)
                    d_b1h1 = rt_piece(c, 1, 1, bsb1, 'D', 0, NBLK)
                    sc_h3 = rt_piece(c, 0, 3, bsb0, 'S', 0, 8)
                    d_b0h3 = rt_piece(c, 0, 3, bsb0, 'D', 8, NBLK)
                    d_b1h3 = rt_piece(c, 1, 3, bsb1, 'D', 0, NBLK)
                    # h1+h3 merged into one 4-strip wave; n=8..15 first so
                    # the late producers (ScalarE's h3 half) gate only the
                    # trailing blocks
                    emit_subwave(c, [(1, 1, [d_b1h1]), (1, 3, [d_b1h3]),
                                     (0, 1, [sc_h1]),
                                     (0, 3, [sc_h3, d_b0h3])],
                                 n_order=list(range(8, NBLK)) + list(range(8)))
                    if c == 1:
                        # evictions: g0/g1 on ScalarE, g3 on DVE in parallel
                        emit_evict(0)
                        emit_evict(2)
                        emit_evict(1)
                        nc.vector.tensor_scalar(
                            sb_out[96:128, :], lp[3][96:128, :],
                            G[96:128, _G_ENCB + 6:_G_ENCB + 7],
                            None, ALU.add)

                nc.sync.dma_start(out[0:64], sb_out[0:64])
                nc.sync.dma_start(out[64:128], sb_out[64:128])
    return nc


def _prep_inputs(uav_feat, task_feat, uw0, ub0, uw1, ub1, uw2, ub2,
                 tw0, tb0, tw1, tb1, tw2, tb2, head_q, fw1, fb1, fw2, fb2):
    f, f16n = np.float32, np.float16
    uav = np.asarray(uav_feat, f)
    task = np.asarray(task_feat, f)
    fw1 = np.asarray(fw1, f)
    fw2 = np.asarray(fw2, f)
    Wq, Wk = fw1[:E], fw1[E:]

    bA = np.zeros((128, _A_TOT), f16n)
    bA[0:32, _A_UW0:_A_UW0 + 128] = np.asarray(uw0, f16n)
    bA[0:32, _A_TW0:_A_TW0 + 128] = np.asarray(tw0, f16n)

    bB = np.zeros((128, _B_TOT), f16n)
    bB[:, _B_UW1:_B_UW1 + 128] = np.asarray(uw1, f16n)
    bB[:, _B_UW2:_B_UW2 + 128] = np.asarray(uw2, f16n)
    bB[:, _B_TW1:_B_TW1 + 128] = np.asarray(tw1, f16n)
    bB[:, _B_TW2:_B_TW2 + 128] = np.asarray(tw2, f16n)

    bC = np.zeros((128, _C_TOT), f16n)
    bC[:, _C_WQK:_C_WQK + 256] = Wq.astype(f16n)
    bC[:, _C_WQK + 256:_C_WQK + 512] = Wk.astype(f16n)
    for c in range(2):
        bC[:, _C_WZ + c * 63 + 31] = fw2[c * 128:(c + 1) * 128, 0].astype(f16n)

    bG = np.zeros((128, _G_TOT), f)
    for i, v in enumerate((ub0, ub1, ub2, tb0, tb1, tb2)):
        bG[:, _G_ENCB + i] = np.asarray(v, f)
    bG[:, _G_ENCB + 6] = np.asarray(fb2, f)[0]
    hq = np.asarray(head_q, f) @ Wq + np.asarray(fb1, f)  # (H, HID)
    for c in range(2):
        for h in range(H):
            bG[:, _G_HQPB + c * 4 + h] = hq[h, c * 128:(c + 1) * 128]

    in_maps = []
    for k in range(NCORES):
        b0 = k * BL
        pk = bA.copy()
        pk[0:32, _A_UAVT:_A_UAVT + BL * U] = \
            uav[b0:b0 + BL].reshape(BL * U, UAV_DIM).T.astype(f16n)
        pk[0:32, _A_TASKT:_A_TASKT + BL * T] = \
            task[b0:b0 + BL].reshape(BL * T, TASK_DIM).T.astype(f16n)
        in_maps.append({"pA": pk, "pB": bB, "pC": bC, "pG": bG})
    return in_maps


def _gather(results):
    outs = []
    for k in range(NCORES):
        r = np.asarray(results[k]["out"], np.float32)  # (128, 512)
        outs.append(r.reshape(BL, H, NBLK, 4, T).reshape(BL, H, U, T))
    return np.concatenate(outs, axis=0)


def kernel(**inputs) -> np.ndarray:
    if "nc" not in _cache:
        _cache["nc"] = _build_nc()
    nc = _cache["nc"]
    in_maps = _prep_inputs(**inputs)
    if os.environ.get("BASS_KERNEL_SIM"):
        # CoreSim can't digest the hand-inserted wait-splitting NoOps; it
        # enforces the multi-wait semantics natively, so run unsplit.
        from concourse.bass_interp import CoreSim
        results = []
        for k in range(NCORES):
            sim = CoreSim(nc)
            for name, arr in in_maps[k].items():
                sim.tensor(name)[:] = arr
            sim.simulate()
            results.append({"out": np.array(sim.tensor("out"))})
    else:
        from concourse.bass_utils import run_bass_kernel_spmd
        if not _cache.get("split"):
            _split_multi_waits(nc)
            _cache["split"] = True
        results = run_bass_kernel_spmd(nc, in_maps, list(range(NCORES))).results
    return _gather(results)
